# revision 33
# baseline (speedup 1.0000x reference)
"""Correlation module kernel for 8 TRN2 NeuronCores.

Reference computation (per batch element n, pure data-parallel over N):
    A_n = X_n @ U_n^T / sqrt(D)          # [L, O]
    W_n = sigmoid(A_n) - 0.5             # = 0.5 * tanh(A_n / 2)
    F_n = W_n @ U_n                      # [L, D]

Shapes: x [L=512, N=64, D=512] f32, upfold [O=512, N=64, D=512] f32.
Sharding: N axis across 8 cores (8 batch elements per core), no comms.

Device kernel (per core, per n):
    MM1:  psum_AT[o, l] = sum_d uT[d, o] * xT[d, l]      (fp16 in, f32 acc)
    ACT:  w[o, l] = tanh(psum_AT * 1/(2*sqrt(D)))        (-> fp16)
    MM2:  psum_F[l, d] = sum_o w[o, l] * (0.5*u)[o, d]   (fp16 in, f32 acc)
    DVE:  f[l, d] = psum_F                               (-> fp16)
    DMA out to y[n, l, d]; host upcasts to f32 and transposes to [L,N,D].

Host pre-arranges per-core inputs as fp16 in the exact layouts the PE
needs (d-major for MM1 operands, o-major for MM2's moving operand), so
the device does zero transposes and minimum HBM traffic (16.8 MB/core).

Schedule notes (measured on HW via neuron-profile / ntff analysis):
 - Fixed runtime overhead: ~7.3us preamble (engine barriers + register
   loads) and ~7.8us epilogue (the runtime individually zeroes all 253
   semaphores, ~115ns each on the straggling PE sequencer).  Both are
   emitted by the NEURON runtime at NEFF load, NOT by walrus (the
   engine .bins inside file.neff end at the tile-exit barrier), so
   neither is controllable from kernel code; exec_time is measured to
   the literal last instruction, so the post-last-matmul store tail
   shifts the epilogue 1:1.
 - The PE clock ramp (HAM) is TIME-based: 2.4GHz arrives ~3.4us after
   sustained PE activity begins (earliest ~7.4us, after the preamble
   barrier).  10 dummy FD=512 warm-up matmuls on scratch SBUF bridge to
   the first loads' arrival (~11-12us); an idle gap OR a load-stall
   during the ramp window RESETS it (measured +1.5-15us), so the bridge
   must not undershoot.  A dummy tanh pre-triggers the 1.3us ACT tanh
   table load.
 - Early in the kernel all 8 cores burst their first loads at once and
   each HWDGE queue sustains only ~90-140 GB/s (steady-state ~220);
   the first packet leaves ~1.6-2.3us after the issue instruction, and
   each DMA issue occupies its engine ~0.65us.  n0's xt halves ride the
   sync ring and ut halves the scalar ring in parallel; the first real
   matmul deliberately gates on a COARSE 512KB (xt[0:2]+ut[0:2]) so
   ~8 matmuls of runway are buffered and the stream never stalls during
   the ramp.  Finer 128KB first chunks start real matmuls ~1-2us
   earlier but stall mid-ramp whenever DMA is slow — measured negative
   EV (gain ~1us best case, loss 2-15us on slow runs).  SWDGE loads
   are far slower still; loads stay on HWDGE.
 - MM1 runs d-major over 4 PSUM banks then closes each o-block early so
   its tanh overlaps remaining matmuls; MM2 runs o-major over 4 more
   banks.  Steady state: 216 ns/matmul (512 rows @ 2.4 GHz, LDWEIGHTS
   fully hidden) — the hard PE floor; the stream runs stall-free.  Some
   runs are HAM/GPIO power throttled to 13/16 duty (262.6 ns pitch);
   that is environmental.
 - y is [NLOC, L, D] so every store writes a contiguous DRAM range (the
   host folds the transpose back to [L, N, D] into the fp32 upcast it
   does anyway).  Stores for n0-n6 ride the gpsimd SWDGE ring (a single
   queue at ~90 GB/s — the num_swdge_queues=4 spread does not apply to
   plain dma_start — but it keeps DMA issue off the ACT/Sync rings and
   comfortably keeps pace with the 6.9us/element compute pitch).  The
   last element closes lb0/lb1 early (lb-major) and lb2/lb3 in
   d-halves, each in its own PSUM tile borrowed from the freed MM1
   banks so each piece's cast+store overlaps the remaining matmuls; its
   stores spread over the sync/scalar HWDGE rings in <=64KB chunks.
   The post-last-matmul tail floor is ~2.4-2.8us: cast(0.3) + DMA
   issue(0.65) + ring start latency(0.7) + packets + completion
   sem(0.5).
"""

import numpy as np

L, O, N, D = 512, 512, 64, 512
NCORES = 8
NLOC = N // NCORES  # 8 batch elements per core
P = 128  # SBUF partitions
DB = D // P  # 4 d-blocks
OB = O // P  # 4 o-blocks
LB = L // P  # 4 l-blocks
# 10 bridges to typical load arrival (~11.9us); the 11th costs ~0.13us
# on fast-DMA runs (loads gate the first real matmul anyway) but covers
# slow-DMA arrivals (~12.5us) that would otherwise open a mid-ramp PE
# gap worth ~1us of HAM penalty.
WARMUP_MMS = 11

_cache = {}


def _build_program():
    import concourse.bass as bass
    import concourse.mybir as mybir
    import concourse.tile as tile
    from concourse import bacc

    FP16 = mybir.dt.float16
    F32 = mybir.dt.float32
    Tanh = mybir.ActivationFunctionType.Tanh
    Copy = mybir.ActivationFunctionType.Copy

    nc = bacc.Bacc(
        "TRN2", target_bir_lowering=False, debug=False, num_swdge_queues=4
    )
    xt_d = nc.declare_dram_parameter("xt", [NLOC, D, L], FP16, isOutput=False)
    ut_d = nc.declare_dram_parameter("ut", [NLOC, D, O], FP16, isOutput=False)
    un_d = nc.declare_dram_parameter("un", [NLOC, O, D], FP16, isOutput=False)
    # y is [NLOC, L, D] (contiguous 512KB per batch element) so store DMA
    # descriptors write fully contiguous DRAM ranges; the host transposes
    # back to [L, N, D] during the fp32 upcast it does anyway.
    y_d = nc.declare_dram_parameter("y", [NLOC, L, D], FP16, isOutput=True)

    s2 = 1.0 / (2.0 * float(np.sqrt(D)))  # tanh half-argument scale

    with tile.TileContext(nc) as tc:
        with (
            tc.tile_pool(name="xt", bufs=NLOC) as xt_pool,
            tc.tile_pool(name="ut", bufs=NLOC) as ut_pool,
            tc.tile_pool(name="un", bufs=NLOC) as un_pool,
            tc.tile_pool(name="w", bufs=2) as w_pool,
            tc.tile_pool(name="fo", bufs=2) as f_pool,
            tc.tile_pool(name="scr", bufs=1) as scr_pool,
            tc.tile_pool(name="psa", bufs=1, space="PSUM") as psa_pool,
            tc.tile_pool(name="psf", bufs=1, space="PSUM") as psf_pool,
        ):
            scr_t = scr_pool.tile([P, L], FP16, tag="scr")
            nc.gpsimd.memset(scr_t[:], 0.0)
            scr2_t = scr_pool.tile([P, 1], FP16, tag="scr2")
            nc.scalar.activation(scr2_t[:], scr_t[:, 0:1], Tanh, scale=s2)
            ps_w = psa_pool.tile([P, L], F32, tag="psa0", name="ps_warm")
            for _ in range(WARMUP_MMS):
                nc.tensor.matmul(
                    ps_w[:], lhsT=scr_t[:, :P], rhs=scr_t[:], start=True, stop=True
                )

            for n in range(NLOC):
                xt_t = xt_pool.tile([P, DB, L], FP16, tag="xt")
                ut_t = ut_pool.tile([P, DB, O], FP16, tag="ut")
                un_t = un_pool.tile([P, OB, D], FP16, tag="un")
                xt_ap = xt_d[n].rearrange("(b p) l -> p b l", p=P)
                ut_ap = ut_d[n].rearrange("(b p) o -> p b o", p=P)
                un_ap = un_d[n].rearrange("(b p) d -> p b d", p=P)
                if n == 0:
                    nc.sync.dma_start(xt_t[:, 0:2, :], xt_ap[:, 0:2, :])
                    nc.scalar.dma_start(ut_t[:, 0:2, :], ut_ap[:, 0:2, :])
                    nc.sync.dma_start(xt_t[:, 2:4, :], xt_ap[:, 2:4, :])
                    nc.scalar.dma_start(ut_t[:, 2:4, :], ut_ap[:, 2:4, :])
                    nc.sync.dma_start(un_t[:], un_ap)
                else:
                    nc.sync.dma_start(xt_t[:, 0:2, :], xt_ap[:, 0:2, :])
                    nc.sync.dma_start(ut_t[:, 0:2, :], ut_ap[:, 0:2, :])
                    nc.sync.dma_start(xt_t[:, 2:4, :], xt_ap[:, 2:4, :])
                    nc.sync.dma_start(ut_t[:, 2:4, :], ut_ap[:, 2:4, :])
                    nc.sync.dma_start(un_t[:], un_ap)

                ps_a = [
                    psa_pool.tile([P, L], F32, tag=f"psa{ob}", name=f"ps_a{ob}")
                    for ob in range(OB)
                ]
                mm1_order = [(db, ob) for db in range(2) for ob in range(OB)]
                mm1_order += [(db, ob) for ob in range(OB) for db in range(2, DB)]
                for db, ob in mm1_order:
                    nc.tensor.matmul(
                        ps_a[ob][:],
                        lhsT=ut_t[:, db, bass.ts(ob, P)],
                        rhs=xt_t[:, db, :],
                        start=(db == 0),
                        stop=(db == DB - 1),
                    )
                w_t = w_pool.tile([P, OB, L], FP16, tag="w")
                for ob in range(OB):
                    nc.scalar.activation(w_t[:, ob, :], ps_a[ob][:], Tanh, scale=s2)

                ps_f = [
                    psf_pool.tile([P, D], F32, tag=f"psf{lb}", name=f"ps_f{lb}")
                    for lb in range(LB)
                ]
                last = n == NLOC - 1
                if last:
                    mm2_order = [
                        (ob, lb) for lb in range(LB - 2) for ob in range(OB)
                    ]
                else:
                    mm2_order = [(ob, lb) for ob in range(OB) for lb in range(LB)]
                for ob, lb in mm2_order:
                    nc.tensor.matmul(
                        ps_f[lb][:],
                        lhsT=w_t[:, ob, bass.ts(lb, P)],
                        rhs=un_t[:, ob, :],
                        start=(ob == 0),
                        stop=(ob == OB - 1),
                    )
                ps_h = None
                if last:
                    h = D // 2
                    ps_h = [
                        psa_pool.tile([P, h], F32, tag=f"psa{i}", name=f"ps_h{i}")
                        for i in range(4)
                    ]
                    for i in range(4):
                        lb = LB - 2 + i // 2
                        for ob in range(OB):
                            nc.tensor.matmul(
                                ps_h[i][:],
                                lhsT=w_t[:, ob, bass.ts(lb, P)],
                                rhs=un_t[:, ob, (i % 2) * h : (i % 2 + 1) * h],
                                start=(ob == 0),
                                stop=(ob == OB - 1),
                            )
                f_t = f_pool.tile([P, LB, D], FP16, tag="f")
                y_ap = y_d[n].rearrange("(b p) d -> p b d", p=P)
                for lb in range(LB):
                    if last:
                        h = D // 2
                        q = D // 4
                        if lb == 0:
                            # closes ~2.1us before the last matmul — rides
                            # the by-now-idle SWDGE so sync's post-T issue
                            # chain shrinks to h0/h2/q_a
                            nc.vector.tensor_copy(f_t[:, lb, :], ps_f[lb][:])
                            nc.gpsimd.dma_start(y_ap[:, lb, :], f_t[:, lb, :])
                        elif lb == 1:
                            nc.scalar.activation(f_t[:, lb, :], ps_f[lb][:], Copy)
                            nc.scalar.dma_start(y_ap[:, lb, :], f_t[:, lb, :])
                        elif lb == 2:
                            nc.vector.tensor_copy(f_t[:, lb, 0:h], ps_h[0][:])
                            nc.sync.dma_start(y_ap[:, lb, 0:h], f_t[:, lb, 0:h])
                            nc.vector.tensor_copy(f_t[:, lb, h:D], ps_h[1][:])
                            nc.scalar.dma_start(y_ap[:, lb, h:D], f_t[:, lb, h:D])
                        else:
                            # both final quarter-casts ride the idle DVE
                            # (ACT is still draining lb1/h1 issue work), and
                            # the scalar-bound quarter casts FIRST so its
                            # store issue starts ~T+0.3 instead of ~T+0.7
                            nc.vector.tensor_copy(f_t[:, lb, 0:h], ps_h[2][:])
                            nc.sync.dma_start(y_ap[:, lb, 0:h], f_t[:, lb, 0:h])
                            nc.vector.tensor_copy(
                                f_t[:, lb, h + q : D], ps_h[3][:, q:h]
                            )
                            nc.scalar.dma_start(
                                y_ap[:, lb, h + q : D], f_t[:, lb, h + q : D]
                            )
                            nc.vector.tensor_copy(
                                f_t[:, lb, h : h + q], ps_h[3][:, 0:q]
                            )
                            nc.sync.dma_start(
                                y_ap[:, lb, h : h + q], f_t[:, lb, h : h + q]
                            )
                        continue
                    if lb % 2 == 0:
                        nc.vector.tensor_copy(f_t[:, lb, :], ps_f[lb][:])
                    else:
                        nc.scalar.activation(f_t[:, lb, :], ps_f[lb][:], Copy)
                    nc.gpsimd.dma_start(y_ap[:, lb, :], f_t[:, lb, :])
    nc.compile()
    return nc


def _prepare_in_maps(x, u):
    f16 = np.float16
    in_maps = []
    for c in range(NCORES):
        ns = slice(c * NLOC, (c + 1) * NLOC)
        xs = x[:, ns, :]  # [L, NLOC, D]
        us = u[:, ns, :]  # [O, NLOC, D]
        in_maps.append(
            {
                "xt": np.ascontiguousarray(xs.transpose(1, 2, 0)).astype(f16),
                "ut": np.ascontiguousarray(us.transpose(1, 2, 0)).astype(f16),
                "un": (0.5 * us.transpose(1, 0, 2)).astype(f16),
            }
        )
    return in_maps


def _run(inputs, trace=False, **spmd_kwargs):
    from concourse.bass_utils import run_bass_kernel_spmd

    x = np.asarray(inputs["x"], dtype=np.float32)
    u = np.asarray(inputs["upfold"], dtype=np.float32)
    assert x.shape == (L, N, D) and u.shape == (O, N, D)

    if "nc" not in _cache:
        _cache["nc"] = _build_program()
    nc = _cache["nc"]

    in_maps = _prepare_in_maps(x, u)
    res = run_bass_kernel_spmd(
        nc, in_maps, core_ids=list(range(NCORES)), trace=trace, **spmd_kwargs
    )
    # device y is [NLOC, L, D]; transpose back while assembling [L, N, D]
    out = np.concatenate(
        [r["y"].transpose(1, 0, 2) for r in res.results], axis=1
    )
    return np.ascontiguousarray(out.astype(np.float32)), res


def kernel(**inputs) -> np.ndarray:
    out, _ = _run(inputs, trace=False)
    return out


# revision 37
# speedup vs baseline: 1.0021x; 1.0021x over previous
"""Correlation module kernel for 8 TRN2 NeuronCores.

Reference computation (per batch element n, pure data-parallel over N):
    A_n = X_n @ U_n^T / sqrt(D)          # [L, O]
    W_n = sigmoid(A_n) - 0.5             # = 0.5 * tanh(A_n / 2)
    F_n = W_n @ U_n                      # [L, D]

Shapes: x [L=512, N=64, D=512] f32, upfold [O=512, N=64, D=512] f32.
Sharding: N axis across 8 cores (8 batch elements per core), no comms.

Device kernel (per core, per n):
    MM1:  psum_AT[o, l] = sum_d uT[d, o] * xT[d, l]      (fp16 in, f32 acc)
    ACT:  w[o, l] = tanh(psum_AT * 1/(2*sqrt(D)))        (-> fp16)
    MM2:  psum_F[l, d] = sum_o w[o, l] * (0.5*u)[o, d]   (fp16 in, f32 acc)
    DVE:  f[l, d] = psum_F                               (-> fp16)
    DMA out to y[n, l, d]; host upcasts to f32 and transposes to [L,N,D].

Host pre-arranges per-core inputs as fp16 in the exact layouts the PE
needs (d-major for MM1 operands, o-major for MM2's moving operand), so
the device does zero transposes and minimum HBM traffic (16.8 MB/core).

Schedule notes (measured on HW via neuron-profile / ntff analysis):
 - Fixed runtime overhead: ~7.3us preamble (engine barriers + register
   loads) and ~7.8us epilogue (the runtime individually zeroes all 253
   semaphores, ~115ns each on the straggling PE sequencer).  Both are
   emitted by the NEURON runtime at NEFF load, NOT by walrus (the
   engine .bins inside file.neff end at the tile-exit barrier), so
   neither is controllable from kernel code; exec_time is measured to
   the literal last instruction, so the post-last-matmul store tail
   shifts the epilogue 1:1.
 - The PE clock ramp (HAM) is TIME-based: 2.4GHz arrives ~3.4us after
   sustained PE activity begins (earliest ~7.4us, after the preamble
   barrier).  10 dummy FD=512 warm-up matmuls on scratch SBUF bridge to
   the first loads' arrival (~11-12us); an idle gap OR a load-stall
   during the ramp window RESETS it (measured +1.5-15us), so the bridge
   must not undershoot.  A dummy tanh pre-triggers the 1.3us ACT tanh
   table load.
 - Early in the kernel all 8 cores burst their first loads at once and
   each HWDGE queue sustains only ~90-140 GB/s (steady-state ~220);
   the first packet leaves ~1.6-2.3us after the issue instruction, and
   each DMA issue occupies its engine ~0.65us.  n0's xt halves ride the
   sync ring and ut halves the scalar ring in parallel; the first real
   matmul deliberately gates on a COARSE 512KB (xt[0:2]+ut[0:2]) so
   ~8 matmuls of runway are buffered and the stream never stalls during
   the ramp.  Finer 128KB first chunks start real matmuls ~1-2us
   earlier but stall mid-ramp whenever DMA is slow — measured negative
   EV (gain ~1us best case, loss 2-15us on slow runs).  SWDGE loads
   are far slower still; loads stay on HWDGE.
 - MM1 runs d-major over 4 PSUM banks then closes each o-block early so
   its tanh overlaps remaining matmuls; MM2 runs o-major over 4 more
   banks.  Steady state: 216 ns/matmul (512 rows @ 2.4 GHz, LDWEIGHTS
   fully hidden) — the hard PE floor; the stream runs stall-free.  Some
   runs are HAM/GPIO power throttled to 13/16 duty (262.6 ns pitch);
   that is environmental.
 - y is [NLOC, L, D] so every store writes a contiguous DRAM range (the
   host folds the transpose back to [L, N, D] into the fp32 upcast it
   does anyway).  Stores for n0-n6 ride the gpsimd SWDGE ring (a single
   queue at ~90 GB/s — the num_swdge_queues=4 spread does not apply to
   plain dma_start — but it keeps DMA issue off the ACT/Sync rings and
   comfortably keeps pace with the 6.9us/element compute pitch).  The
   last element closes lb0/lb1 early (lb-major) and lb2/lb3 in
   d-halves, each in its own PSUM tile borrowed from the freed MM1
   banks so each piece's cast+store overlaps the remaining matmuls; its
   stores spread over the sync/scalar HWDGE rings in <=64KB chunks.
   The post-last-matmul tail floor is ~2.4-2.8us: cast(0.3) + DMA
   issue(0.65) + ring start latency(0.7) + packets + completion
   sem(0.5).
"""

import numpy as np

L, O, N, D = 512, 512, 64, 512
NCORES = 8
NLOC = N // NCORES  # 8 batch elements per core
P = 128  # SBUF partitions
DB = D // P  # 4 d-blocks
OB = O // P  # 4 o-blocks
LB = L // P  # 4 l-blocks
# 10 bridges to typical load arrival (~11.9us); the 11th costs ~0.13us
# on fast-DMA runs (loads gate the first real matmul anyway) but covers
# slow-DMA arrivals (~12.5us) that would otherwise open a mid-ramp PE
# gap worth ~1us of HAM penalty.
WARMUP_MMS = 11

_cache = {}


def _build_program():
    import concourse.bass as bass
    import concourse.mybir as mybir
    import concourse.tile as tile
    from concourse import bacc

    FP16 = mybir.dt.float16
    F32 = mybir.dt.float32
    Tanh = mybir.ActivationFunctionType.Tanh
    Copy = mybir.ActivationFunctionType.Copy

    nc = bacc.Bacc(
        "TRN2", target_bir_lowering=False, debug=False, num_swdge_queues=4
    )
    xt_d = nc.declare_dram_parameter("xt", [NLOC, D, L], FP16, isOutput=False)
    ut_d = nc.declare_dram_parameter("ut", [NLOC, D, O], FP16, isOutput=False)
    un_d = nc.declare_dram_parameter("un", [NLOC, O, D], FP16, isOutput=False)
    # y is [NLOC, L, D] (contiguous 512KB per batch element) so store DMA
    # descriptors write fully contiguous DRAM ranges; the host transposes
    # back to [L, N, D] during the fp32 upcast it does anyway.
    y_d = nc.declare_dram_parameter("y", [NLOC, L, D], FP16, isOutput=True)

    s2 = 1.0 / (2.0 * float(np.sqrt(D)))  # tanh half-argument scale

    # warm-up scratch as a RAW sbuf tensor (not a pool tile, which would
    # require a producer): its memset runs in the Bass preamble, so the
    # PE's first warm-up LDWEIGHTS has no cross-engine dependency and the
    # HAM clock ramp starts ~0.7us earlier.  Warm-up output is discarded,
    # so a memset/read race is benign.
    scr_h = nc.alloc_sbuf_tensor("scr_warm", [P, L], FP16)
    nc.gpsimd.memset(scr_h.ap(), 0.0)

    with tile.TileContext(nc) as tc:
        with (
            tc.tile_pool(name="xt", bufs=NLOC) as xt_pool,
            tc.tile_pool(name="ut", bufs=NLOC) as ut_pool,
            tc.tile_pool(name="un", bufs=NLOC) as un_pool,
            tc.tile_pool(name="w", bufs=2) as w_pool,
            tc.tile_pool(name="fo", bufs=2) as f_pool,
            tc.tile_pool(name="scr", bufs=1) as scr_pool,
            tc.tile_pool(name="psa", bufs=1, space="PSUM") as psa_pool,
            tc.tile_pool(name="psf", bufs=1, space="PSUM") as psf_pool,
        ):
            scr_t = scr_h.ap()
            scr2_t = scr_pool.tile([P, 1], FP16, tag="scr2")
            nc.scalar.activation(scr2_t[:], scr_t[:, 0:1], Tanh, scale=s2)
            ps_w = psa_pool.tile([P, L], F32, tag="psa0", name="ps_warm")
            for _ in range(WARMUP_MMS):
                nc.tensor.matmul(
                    ps_w[:], lhsT=scr_t[:, :P], rhs=scr_t[:], start=True, stop=True
                )

            for n in range(NLOC):
                xt_t = xt_pool.tile([P, DB, L], FP16, tag="xt")
                ut_t = ut_pool.tile([P, DB, O], FP16, tag="ut")
                un_t = un_pool.tile([P, OB, D], FP16, tag="un")
                xt_ap = xt_d[n].rearrange("(b p) l -> p b l", p=P)
                ut_ap = ut_d[n].rearrange("(b p) o -> p b o", p=P)
                un_ap = un_d[n].rearrange("(b p) d -> p b d", p=P)
                if n == 0:
                    nc.sync.dma_start(xt_t[:, 0:2, :], xt_ap[:, 0:2, :])
                    nc.scalar.dma_start(ut_t[:, 0:2, :], ut_ap[:, 0:2, :])
                    nc.sync.dma_start(xt_t[:, 2:4, :], xt_ap[:, 2:4, :])
                    nc.scalar.dma_start(ut_t[:, 2:4, :], ut_ap[:, 2:4, :])
                    nc.sync.dma_start(un_t[:], un_ap)
                else:
                    nc.sync.dma_start(xt_t[:, 0:2, :], xt_ap[:, 0:2, :])
                    nc.sync.dma_start(ut_t[:, 0:2, :], ut_ap[:, 0:2, :])
                    nc.sync.dma_start(xt_t[:, 2:4, :], xt_ap[:, 2:4, :])
                    nc.sync.dma_start(ut_t[:, 2:4, :], ut_ap[:, 2:4, :])
                    nc.sync.dma_start(un_t[:], un_ap)

                ps_a = [
                    psa_pool.tile([P, L], F32, tag=f"psa{ob}", name=f"ps_a{ob}")
                    for ob in range(OB)
                ]
                mm1_order = [(db, ob) for db in range(2) for ob in range(OB)]
                mm1_order += [(db, ob) for ob in range(OB) for db in range(2, DB)]
                for db, ob in mm1_order:
                    nc.tensor.matmul(
                        ps_a[ob][:],
                        lhsT=ut_t[:, db, bass.ts(ob, P)],
                        rhs=xt_t[:, db, :],
                        start=(db == 0),
                        stop=(db == DB - 1),
                    )
                w_t = w_pool.tile([P, OB, L], FP16, tag="w")
                for ob in range(OB):
                    nc.scalar.activation(w_t[:, ob, :], ps_a[ob][:], Tanh, scale=s2)

                ps_f = [
                    psf_pool.tile([P, D], F32, tag=f"psf{lb}", name=f"ps_f{lb}")
                    for lb in range(LB)
                ]
                last = n == NLOC - 1
                if last:
                    mm2_order = [
                        (ob, lb) for lb in range(LB - 2) for ob in range(OB)
                    ]
                else:
                    mm2_order = [(ob, lb) for ob in range(OB) for lb in range(LB)]
                for ob, lb in mm2_order:
                    nc.tensor.matmul(
                        ps_f[lb][:],
                        lhsT=w_t[:, ob, bass.ts(lb, P)],
                        rhs=un_t[:, ob, :],
                        start=(ob == 0),
                        stop=(ob == OB - 1),
                    )
                ps_h = None
                if last:
                    h = D // 2
                    ps_h = [
                        psa_pool.tile([P, h], F32, tag=f"psa{i}", name=f"ps_h{i}")
                        for i in range(4)
                    ]
                    for i in range(4):
                        lb = LB - 2 + i // 2
                        for ob in range(OB):
                            nc.tensor.matmul(
                                ps_h[i][:],
                                lhsT=w_t[:, ob, bass.ts(lb, P)],
                                rhs=un_t[:, ob, (i % 2) * h : (i % 2 + 1) * h],
                                start=(ob == 0),
                                stop=(ob == OB - 1),
                            )
                f_t = f_pool.tile([P, LB, D], FP16, tag="f")
                y_ap = y_d[n].rearrange("(b p) d -> p b d", p=P)
                for lb in range(LB):
                    if last:
                        h = D // 2
                        q = D // 4
                        if lb == 0:
                            # closes ~2.1us before the last matmul — rides
                            # the by-now-idle SWDGE so sync's post-T issue
                            # chain shrinks to h0/h2/q_a
                            nc.vector.tensor_copy(f_t[:, lb, :], ps_f[lb][:])
                            nc.gpsimd.dma_start(y_ap[:, lb, :], f_t[:, lb, :])
                        elif lb == 1:
                            nc.scalar.activation(f_t[:, lb, :], ps_f[lb][:], Copy)
                            nc.scalar.dma_start(y_ap[:, lb, :], f_t[:, lb, :])
                        elif lb == 2:
                            nc.vector.tensor_copy(f_t[:, lb, 0:h], ps_h[0][:])
                            nc.sync.dma_start(y_ap[:, lb, 0:h], f_t[:, lb, 0:h])
                            nc.vector.tensor_copy(f_t[:, lb, h:D], ps_h[1][:])
                            nc.scalar.dma_start(y_ap[:, lb, h:D], f_t[:, lb, h:D])
                        else:
                            # both final quarter-casts ride the idle DVE
                            # (ACT is still draining lb1/h1 issue work), and
                            # the scalar-bound quarter casts FIRST so its
                            # store issue starts ~T+0.3 instead of ~T+0.7
                            nc.vector.tensor_copy(f_t[:, lb, 0:h], ps_h[2][:])
                            nc.sync.dma_start(y_ap[:, lb, 0:h], f_t[:, lb, 0:h])
                            nc.vector.tensor_copy(
                                f_t[:, lb, h + q : D], ps_h[3][:, q:h]
                            )
                            nc.scalar.dma_start(
                                y_ap[:, lb, h + q : D], f_t[:, lb, h + q : D]
                            )
                            nc.vector.tensor_copy(
                                f_t[:, lb, h : h + q], ps_h[3][:, 0:q]
                            )
                            nc.sync.dma_start(
                                y_ap[:, lb, h : h + q], f_t[:, lb, h : h + q]
                            )
                        continue
                    if lb % 2 == 0:
                        nc.vector.tensor_copy(f_t[:, lb, :], ps_f[lb][:])
                    else:
                        nc.scalar.activation(f_t[:, lb, :], ps_f[lb][:], Copy)
                    nc.gpsimd.dma_start(y_ap[:, lb, :], f_t[:, lb, :])
    nc.compile()
    return nc


def _prepare_in_maps(x, u):
    f16 = np.float16
    in_maps = []
    for c in range(NCORES):
        ns = slice(c * NLOC, (c + 1) * NLOC)
        xs = x[:, ns, :]  # [L, NLOC, D]
        us = u[:, ns, :]  # [O, NLOC, D]
        in_maps.append(
            {
                "xt": np.ascontiguousarray(xs.transpose(1, 2, 0)).astype(f16),
                "ut": np.ascontiguousarray(us.transpose(1, 2, 0)).astype(f16),
                "un": (0.5 * us.transpose(1, 0, 2)).astype(f16),
            }
        )
    return in_maps


def _run(inputs, trace=False, **spmd_kwargs):
    from concourse.bass_utils import run_bass_kernel_spmd

    x = np.asarray(inputs["x"], dtype=np.float32)
    u = np.asarray(inputs["upfold"], dtype=np.float32)
    assert x.shape == (L, N, D) and u.shape == (O, N, D)

    if "nc" not in _cache:
        _cache["nc"] = _build_program()
    nc = _cache["nc"]

    in_maps = _prepare_in_maps(x, u)
    res = run_bass_kernel_spmd(
        nc, in_maps, core_ids=list(range(NCORES)), trace=trace, **spmd_kwargs
    )
    # device y is [NLOC, L, D]; transpose back while assembling [L, N, D]
    out = np.concatenate(
        [r["y"].transpose(1, 0, 2) for r in res.results], axis=1
    )
    return np.ascontiguousarray(out.astype(np.float32)), res


def kernel(**inputs) -> np.ndarray:
    out, _ = _run(inputs, trace=False)
    return out
